# revision 7
# baseline (speedup 1.0000x reference)
"""Pointer-network (enc LSTM -> dec LSTM + attention) Trainium2 Bass kernel.

Sharding: pure data-parallel over batch B=256 across 8 NeuronCores (32/core).
Everything SBUF-resident per core; sequential scan over L stays on-core.

v2 redesign vs baseline:
- LSTM gate matmuls FLIPPED: hidden state is the stationary operand
  (lhsT [128,32], LDW ~27ns) and the weight matrices stream through the
  moving-rhs port at N=512 (213ns/MM at 2.4GHz). Gates land as [32b, 2048g]
  in PSUM (4 banks); pointwise runs in [32, 512]-slice layout; h returns to
  [hsub, hc, b] via 4 PE transposes per step.
- Exact hi/lo bf16 split of the integer inputs (x = hi + lo, both exact in
  bf16) folded into the gate accumulation as a 3-partition matmul.
- Zero in-loop DMAs: teacher inputs preloaded to SBUF, sliced per step.
- Scores / ctx keep the per-l / per-(b,hc) stationary structure (forced by
  layout) but everything else sheds ~2.5x PE instructions per step.
- Deferred log-softmax via DRAM round trip (avoids ACT table switches);
  post-pass batched 16 steps per DMA.

Per-core layouts (p = partition dim):
  hT      [128 hsub, 4 hc, 32 b]      state for matmul lhsT
  h_sb/c  [32 b, 512 h]               state for pointwise
  E_sb    [128 hsub, 128 l, 128 (hc*32+b)] = enc_out @ w1.T (bf16)
  T_sb    same layout, tanh(E + q) per decode step
  H_all   [128 hsub, 4 hc, 32 b, 128 l]   encoder outputs
  E0T     [128 l, 32 b, 4 hc, 128 hsub]   enc_out transposed (ctx lhsT)
  g_ps    [32 b, 4 gc, 512]           PSUM gates, i/f/g/o = gc 0..3
"""

import os
import sys

import numpy as np

for _p in ("/opt/trn_rl_repo", os.environ.get("TRN_RL_REPO", "")):
    if _p and _p not in sys.path and os.path.isdir(_p):
        sys.path.insert(0, _p)

import ml_dtypes

bf16 = ml_dtypes.bfloat16

B, L, H = 256, 128, 512
NCORES = 8
BL = B // NCORES  # 32
HC = H // 128     # 4
G = 4 * H         # 2048

_cache = {}


def _build_nc(enc_steps=L, dec_steps=L, unroll=8, static_loops=False):
    import concourse.bass as bass
    import concourse.bacc as bacc
    import concourse.tile as tile
    from concourse import mybir
    from concourse.masks import make_identity

    AFT = mybir.ActivationFunctionType
    ALU = mybir.AluOpType
    f32 = mybir.dt.float32
    b16 = mybir.dt.bfloat16

    nc = bacc.Bacc("TRN2", target_bir_lowering=False, debug=False)

    xa_d = nc.dram_tensor("xa", [3, L * BL], b16, kind="ExternalInput").ap()
    dt_d = nc.dram_tensor("dt", [3, L * BL], b16, kind="ExternalInput").ap()
    ewb_d = nc.dram_tensor("ewb", [3, G], b16, kind="ExternalInput").ap()
    dwb_d = nc.dram_tensor("dwb", [3, G], b16, kind="ExternalInput").ap()
    ewhT_d = nc.dram_tensor("ewhT", [128, HC, G], b16, kind="ExternalInput").ap()
    dwhT_d = nc.dram_tensor("dwhT", [128, HC, G], b16, kind="ExternalInput").ap()
    dwiT_d = nc.dram_tensor("dwiT", [128, HC, G], b16, kind="ExternalInput").ap()
    w1T_d = nc.dram_tensor("w1T", [128, HC, H], b16, kind="ExternalInput").ap()
    w2T_d = nc.dram_tensor("w2T", [128, HC, H], b16, kind="ExternalInput").ap()
    v4_d = nc.dram_tensor("v4", [128, HC], b16, kind="ExternalInput").ap()
    outp_d = nc.dram_tensor("outp", [BL, L, L], f32, kind="ExternalOutput").ap()
    sstore_d = nc.dram_tensor("sstore", [L, BL, L], f32).ap()

    with tile.TileContext(nc) as tc, tc.tile_pool(name="perm", bufs=1) as perm:
        E_sb = perm.tile([128, L, 128], b16)
        E0T = perm.tile([128, BL, HC, 128], b16)
        w1T = perm.tile([128, HC, H], b16)
        w2T = perm.tile([128, HC, H], b16)
        v4 = perm.tile([128, HC], b16)
        eye = perm.tile([128, 128], f32)
        eye16 = perm.tile([128, 128], b16)
        hT = perm.tile([128, HC, BL], b16)
        c_sb = perm.tile([BL, H], b16)
        xa_all = perm.tile([3, L * BL], b16)
        dt_all = perm.tile([3, L * BL], b16)
        s_all = perm.tile([BL, L], f32)
        negm_all = perm.tile([BL, L], f32)

        nc.sync.dma_start(w1T, w1T_d)
        nc.sync.dma_start(w2T, w2T_d)
        nc.sync.dma_start(v4, v4_d)
        nc.sync.dma_start(xa_all, xa_d)
        nc.sync.dma_start(dt_all, dt_d)
        make_identity(nc, eye)
        nc.vector.tensor_copy(eye16, eye)
        nc.vector.memset(hT, 0.0)
        nc.vector.memset(c_sb, 0.0)
        nc.vector.memset(s_all, 1.0)
        nc.vector.memset(negm_all, 0.0)
        if enc_steps < L:
            nc.vector.memset(E_sb, 0.0)

        def lstm_pointwise(work, g_ps, n_parts=4):
            """g_ps [32, 4, 512] PSUM -> update h_sb (bf16 [32,512]), c_sb.

            sigmoid via tanh: sig(x) = (tanh(x/2)+1)/2 so the whole loop
            stays on one ACT table set.  Returns h_sb.
            """
            ti = work.tile([BL, H], b16, tag="ti")
            tf = work.tile([BL, H], b16, tag="tf")
            tg = work.tile([BL, H], b16, tag="tg")
            to = work.tile([BL, H], b16, tag="to")
            nc.scalar.activation(ti, g_ps[:, 0, :], AFT.Tanh, scale=0.5)
            nc.scalar.activation(tf, g_ps[:, 1, :], AFT.Tanh, scale=0.5)
            nc.scalar.activation(tg, g_ps[:, 2, :], AFT.Tanh, scale=1.0)
            nc.scalar.activation(to, g_ps[:, 3, :], AFT.Tanh, scale=0.5)
            sgi = work.tile([BL, H], b16, tag="sgi")
            sgf = work.tile([BL, H], b16, tag="sgf")
            sgo = work.tile([BL, H], b16, tag="sgo")
            for sg, t_ in ((sgi, ti), (sgf, tf), (sgo, to)):
                nc.vector.tensor_scalar(out=sg, in0=t_, scalar1=1.0,
                                        scalar2=0.5, op0=ALU.add, op1=ALU.mult)
            u = work.tile([BL, H], b16, tag="u")
            v = work.tile([BL, H], b16, tag="v")
            nc.vector.tensor_mul(u, sgi, tg)
            nc.vector.tensor_mul(v, sgf, c_sb)
            nc.vector.tensor_add(c_sb, u, v)
            thc = work.tile([BL, H], b16, tag="thc")
            nc.scalar.activation(thc, c_sb, AFT.Tanh, scale=1.0)
            h_sb = work.tile([BL, H], b16, tag="hsb")
            nc.vector.tensor_mul(h_sb, sgo, thc)
            return h_sb

        def h_to_hT(work, pst, h_sb):
            """h_sb [32, 512] -> hT [128, hc, 32] via 4 PE transposes."""
            for hc in range(HC):
                tr_ps = pst.tile([128, BL], b16, tag="tr")
                nc.tensor.transpose(tr_ps, h_sb[:, hc * 128:(hc + 1) * 128],
                                    eye16[0:BL, 0:BL])
                nc.vector.tensor_copy(hT[:, hc, :], tr_ps)

        # ---------------- encoder ----------------
        with tc.tile_pool(name="encp", bufs=1) as encp, \
             tc.tile_pool(name="encw", bufs=2) as encw, \
             tc.tile_pool(name="encr", bufs=2) as encr, \
             tc.tile_pool(name="psg", bufs=1, space="PSUM") as psg, \
             tc.tile_pool(name="pse", bufs=1, space="PSUM") as pse, \
             tc.tile_pool(name="pst", bufs=2, space="PSUM") as pst:
            ewhT = encp.tile([128, HC, G], b16)
            ewb = encp.tile([3, G], b16)
            H_all = encp.tile([128, HC, BL, L], b16)
            nc.sync.dma_start(ewhT, ewhT_d)
            nc.sync.dma_start(ewb, ewb_d)
            if enc_steps < L:
                nc.vector.memset(H_all, 0.0)

            def enc_body(iv):
                xa_t = encr.tile([3, BL], b16, tag="xa")
                nc.vector.tensor_copy(xa_t, xa_all[:, bass.ds(iv * BL, BL)])
                g_ps = psg.tile([BL, HC, H], mybir.dt.float32, tag="gps")
                for kc in range(HC):
                    for gc in range(4):
                        nc.tensor.matmul(
                            g_ps[:, gc, :], lhsT=hT[:, kc, :],
                            rhs=ewhT[:, kc, gc * H:(gc + 1) * H],
                            start=(kc == 0), stop=False)
                for gc in range(4):
                    nc.tensor.matmul(
                        g_ps[:, gc, :], lhsT=xa_t,
                        rhs=ewb[:, gc * H:(gc + 1) * H],
                        start=False, stop=True)
                h_sb = lstm_pointwise(encw, g_ps)
                h_to_hT(encw, pst, h_sb)
                nc.vector.tensor_copy(H_all[:, :, :, bass.ds(iv, 1)],
                                      hT.unsqueeze(-1))
                e_ps = pse.tile([128, HC, BL], mybir.dt.float32, tag="eps")
                for pc in range(HC):
                    for kc in range(HC):
                        nc.tensor.matmul(
                            e_ps[:, pc, :],
                            lhsT=w1T[:, kc, pc * 128:(pc + 1) * 128],
                            rhs=hT[:, kc, :], start=(kc == 0), stop=(kc == 3))
                dst = E_sb[:, bass.ds(iv, 1), :]
                nc.vector.tensor_copy(
                    dst, e_ps.rearrange("p a b -> p (a b)").unsqueeze(1))

            if static_loops:
                for _i in range(enc_steps):
                    enc_body(_i)
            else:
                tc.For_i_unrolled(0, enc_steps, 1, enc_body, max_unroll=unroll)

            # H_all [hsub, hc, b, l] -> E0T [l, b, hc, hsub] via PE transposes
            for hc in range(HC):
                for b in range(BL):
                    tr_ps = pst.tile([128, 128], b16, tag="tr")
                    nc.tensor.transpose(tr_ps, H_all[:, hc, b, :], eye16)
                    nc.vector.tensor_copy(E0T[:, b, hc, :], tr_ps)

        # ---------------- decoder ----------------
        with tc.tile_pool(name="decp", bufs=1) as decp, \
             tc.tile_pool(name="decw", bufs=2) as decw, \
             tc.tile_pool(name="decx", bufs=2) as decx, \
             tc.tile_pool(name="decr", bufs=2) as decr, \
             tc.tile_pool(name="psqz", bufs=1, space="PSUM") as psqz, \
             tc.tile_pool(name="psc", bufs=1, space="PSUM") as psc, \
             tc.tile_pool(name="psg2", bufs=1, space="PSUM") as psg2, \
             tc.tile_pool(name="pst2", bufs=2, space="PSUM") as pst2:
            dwhT = decp.tile([128, HC, G], b16)
            dwiT = decp.tile([128, HC, G], b16)
            dwb = decp.tile([3, G], b16)
            T_sb = decp.tile([128, L, 128], b16)
            nc.sync.dma_start(dwhT, dwhT_d)
            nc.sync.dma_start(dwiT, dwiT_d)
            nc.sync.dma_start(dwb, dwb_d)

            def dec_body(iv):
                dt_t = decr.tile([3, BL], b16, tag="dt")
                nc.vector.tensor_copy(dt_t, dt_all[:, bass.ds(iv * BL, BL)])
                # q.T [hsub, hc, b] from pre-update h
                q_ps = psqz.tile([128, L, HC], mybir.dt.float32, tag="qz")
                qv = q_ps.rearrange("p a b -> p (a b)")
                for pc in range(HC):
                    for kc in range(HC):
                        nc.tensor.matmul(
                            qv[:, pc * BL:(pc + 1) * BL],
                            lhsT=w2T[:, kc, pc * 128:(pc + 1) * 128],
                            rhs=hT[:, kc, :], start=(kc == 0), stop=(kc == 3))
                qT = decw.tile([128, HC, BL], b16, tag="qT")
                nc.vector.tensor_copy(
                    qT.rearrange("p a b -> p (a b)"), qv[:, 0:H // HC])
                # early gate accumulation: h part + teacher input part
                g_ps = psg2.tile([BL, HC, H], mybir.dt.float32, tag="gps2")
                for kc in range(HC):
                    for gc in range(4):
                        nc.tensor.matmul(
                            g_ps[:, gc, :], lhsT=hT[:, kc, :],
                            rhs=dwhT[:, kc, gc * H:(gc + 1) * H],
                            start=(kc == 0), stop=False)
                for gc in range(4):
                    nc.tensor.matmul(
                        g_ps[:, gc, :], lhsT=dt_t,
                        rhs=dwb[:, gc * H:(gc + 1) * H],
                        start=False, stop=False)
                # X = E + q (broadcast over l), T = tanh(X): 4 l-blocks
                qflat = qT.rearrange("p a b -> p (a b)")
                for blk in range(4):
                    X_blk = decx.tile([128, 32, 128], b16, tag="X")
                    q_b = bass.AP(tensor=qflat.tensor, offset=qflat.offset,
                                  ap=[qflat.ap[0], [0, 32], qflat.ap[1]])
                    nc.vector.tensor_add(
                        X_blk, E_sb[:, blk * 32:(blk + 1) * 32, :], q_b)
                    nc.scalar.activation(
                        T_sb[:, blk * 32:(blk + 1) * 32, :], X_blk,
                        AFT.Tanh, scale=1.0)
                # scores: Z[l] = T_l.T @ v4, diagonal extract
                Z_ps = psqz.tile([128, L, HC], mybir.dt.float32, tag="qz")
                for l in range(L):
                    nc.tensor.matmul(Z_ps[:, l, :], lhsT=T_sb[:, l, :],
                                     rhs=v4, start=True, stop=True)
                S_sb = decw.tile([BL, L], mybir.dt.float32, tag="S")
                nc.vector.tensor_copy(S_sb, Z_ps[0:32, :, 0])
                nc.vector.tensor_add(S_sb, S_sb, Z_ps[32:64, :, 1])
                nc.vector.tensor_add(S_sb, S_sb, Z_ps[64:96, :, 2])
                nc.vector.tensor_add(S_sb, S_sb, Z_ps[96:128, :, 3])
                # softmax pieces (log deferred to post-pass)
                negm = decw.tile([BL, 1], mybir.dt.float32, tag="negm")
                nc.vector.tensor_reduce(out=negm, in_=S_sb,
                                        axis=mybir.AxisListType.X,
                                        op=ALU.max, negate=True)
                e_sb = decw.tile([BL, L], mybir.dt.float32, tag="e")
                nc.scalar.activation(e_sb, S_sb, AFT.Exp, bias=negm, scale=1.0)
                s_t = decw.tile([BL, 1], mybir.dt.float32, tag="s")
                nc.vector.tensor_reduce(out=s_t, in_=e_sb,
                                        axis=mybir.AxisListType.X, op=ALU.add)
                nc.vector.tensor_copy(negm_all[:, bass.ds(iv, 1)], negm)
                nc.vector.tensor_copy(s_all[:, bass.ds(iv, 1)], s_t)
                r = decw.tile([BL, 1], mybir.dt.float32, tag="r")
                nc.vector.reciprocal(r, s_t)
                a_sb = decw.tile([BL, L], mybir.dt.float32, tag="a")
                nc.vector.tensor_scalar_mul(a_sb, e_sb, r)
                nc.sync.dma_start(sstore_d[bass.ds(iv, 1), :, :], S_sb)
                # context via per-(b,hc) stationary matmuls
                aT_ps = pst2.tile([128, BL], mybir.dt.float32, tag="tr")
                nc.tensor.transpose(aT_ps, a_sb, eye[0:BL, 0:BL])
                aT = decw.tile([128, BL], b16, tag="aTs")
                nc.vector.tensor_copy(aT, aT_ps)
                ctx_ps = psc.tile([128, HC, BL], mybir.dt.float32, tag="cps")
                for b in range(BL):
                    for hc in range(HC):
                        nc.tensor.matmul(ctx_ps[:, hc, b:b + 1],
                                         lhsT=E0T[:, b, hc, :],
                                         rhs=aT[:, b:b + 1],
                                         start=True, stop=True)
                xT = decw.tile([128, HC, BL], b16, tag="xT")
                nc.vector.tensor_copy(xT, ctx_ps)
                # late gate accumulation: context part
                for kc in range(HC):
                    for gc in range(4):
                        nc.tensor.matmul(
                            g_ps[:, gc, :], lhsT=xT[:, kc, :],
                            rhs=dwiT[:, kc, gc * H:(gc + 1) * H],
                            start=False, stop=(kc == 3))
                h_sb = lstm_pointwise(decw, g_ps)
                h_to_hT(decw, pst2, h_sb)

            if static_loops:
                for _i in range(dec_steps):
                    dec_body(_i)
            else:
                tc.For_i_unrolled(0, dec_steps, 1, dec_body, max_unroll=unroll)

        # ---------------- deferred log-softmax, batched ----------------
        with tc.tile_pool(name="post", bufs=3) as post, \
             tc.tile_pool(name="postc", bufs=1) as postc:
            lnm = postc.tile([BL, L], mybir.dt.float32)
            nc.scalar.activation(lnm, s_all, AFT.Ln, scale=1.0)
            lnm2 = postc.tile([BL, L], mybir.dt.float32)
            nc.vector.tensor_sub(lnm2, lnm, negm_all)
            CH = 16
            for t0 in range(0, dec_steps, CH):
                n = min(CH, dec_steps - t0)
                S_t = post.tile([BL, CH, L], mybir.dt.float32, tag="St")
                nc.sync.dma_start(
                    S_t[:, 0:n, :],
                    bass.AP(tensor=sstore_d.tensor,
                            offset=sstore_d.offset + t0 * BL * L,
                            ap=[[L, BL], [BL * L, n], [1, L]]))
                o_t = post.tile([BL, CH, L], mybir.dt.float32, tag="ot")
                for j in range(n):
                    nc.vector.tensor_scalar(
                        out=o_t[:, j, :], in0=S_t[:, j, :],
                        scalar1=lnm2[:, t0 + j:t0 + j + 1], scalar2=None,
                        op0=ALU.subtract)
                nc.sync.dma_start(outp_d[:, t0:t0 + n, :], o_t[:, 0:n, :])

    nc.finalize()
    return nc


def _hi_lo_ones(vals):
    """vals [N] float (integers 0..999) -> [3, N] bf16 rows (hi, lo, 1)."""
    v = vals.astype(np.float32)
    hi = np.floor(v / 8.0) * 8.0
    lo = v - hi
    out = np.empty((3, v.shape[0]), np.float32)
    out[0] = hi
    out[1] = lo
    out[2] = 1.0
    return out.astype(bf16)


def _prep_weights(enc_Wi, enc_Wh, enc_b, dec_Wi, dec_Wh, dec_b, w1, w2, vt):
    """Host-side weight repack (shared across cores)."""
    f = np.float32

    def chunkT(W):  # [4H, H] -> [128, HC, 4H]: out[p, kc, g] = W[g, kc*128+p]
        Wt = np.ascontiguousarray(W.astype(f).T)          # [H, 4H]
        return Wt.reshape(HC, 128, 4 * H).transpose(1, 0, 2).astype(bf16)

    def chunkT_sq(W):  # [H, H] -> [128, HC, H]
        Wt = np.ascontiguousarray(W.astype(f).T)
        return Wt.reshape(HC, 128, H).transpose(1, 0, 2).astype(bf16)

    ewb = np.stack([enc_Wi.astype(f)[:, 0], enc_Wi.astype(f)[:, 0],
                    enc_b.astype(f)]).astype(bf16)
    dwb = np.stack([dec_Wi.astype(f)[:, H], dec_Wi.astype(f)[:, H],
                    dec_b.astype(f)]).astype(bf16)
    return {
        "ewb": ewb, "dwb": dwb,
        "ewhT": chunkT(enc_Wh), "dwhT": chunkT(dec_Wh),
        "dwiT": chunkT(dec_Wi[:, :H]),
        "w1T": chunkT_sq(w1), "w2T": chunkT_sq(w2),
        "v4": vt.astype(f)[0].reshape(HC, 128).T.astype(bf16).copy(),
    }


def kernel(xs, x_lens, argsort_xs, enc_Wi, enc_Wh, enc_b,
           dec_Wi, dec_Wh, dec_b, w1, w2, vt):
    from concourse.bass_utils import run_bass_kernel_spmd

    if "nc" not in _cache:
        _cache["nc"] = _build_nc()
    nc = _cache["nc"]

    wmap = _prep_weights(enc_Wi, enc_Wh, enc_b, dec_Wi, dec_Wh, dec_b,
                         w1, w2, vt)
    xs_f = xs.astype(np.float32)
    D = np.concatenate(
        [np.zeros((B, 1), np.float32),
         np.take_along_axis(xs_f, argsort_xs[:, :-1].astype(np.int64), axis=1)],
        axis=1)  # [B, L] teacher-forced decoder inputs

    in_maps = []
    for c in range(NCORES):
        sl = slice(c * BL, (c + 1) * BL)
        m = dict(wmap)
        m["xa"] = _hi_lo_ones(xs_f[sl].T.reshape(-1))  # [3, l*BL+b]
        m["dt"] = _hi_lo_ones(D[sl].T.reshape(-1))
        in_maps.append(m)

    _cache["in_maps"] = in_maps
    res = run_bass_kernel_spmd(nc, in_maps, core_ids=list(range(NCORES)))
    out = np.concatenate([res.results[c]["outp"] for c in range(NCORES)], axis=0)
    return np.ascontiguousarray(out.astype(np.float32))
